# revision 49
# baseline (speedup 1.0000x reference)
"""Attentional Factorization Machine kernel for 8 Trainium2 NeuronCores.

Data-parallel over batch: 1024 rows -> 128 per core. Per core, per batch row:
  mm1: aw = W @ hp (hp = all 780 field-pair products, built on DVE)
  relu: ACT or DVE (split to balance engine load), bias fused
  scores + p_w-projection: one-hot stationary matmuls, issued as 4-wide waves
    across 4 distinct PE column groups so all four stream concurrently
  softmax over pairs + weighted combine on-chip in [128, 780] layout.

Structure notes:
- g-waves are emitted before sc-waves: g only needs hp (ready early), sc
  needs relu; this keeps the list scheduler from scrambling the group
  rotation that gives 4-way column concurrency.
- hp tensor_tensor builds for block t are interleaved into block t-1's row
  loop so the DVE queue never has a bulk burst blocking a due relu.
- HAM pre-warm: N=256 matmuls on a memset tile keep the PE busy from ~5us
  so the clock gate is open when real matmuls start.
"""
import sys
for _p in ("/opt/trn_rl_repo",):
    if _p not in sys.path:
        sys.path.insert(0, _p)

import numpy as np

import concourse.bass as bass
import concourse.bacc as bacc
import concourse.mybir as mybir
import concourse.tile as tile

F32 = mybir.dt.float32
F16 = mybir.dt.float16
AF = mybir.ActivationFunctionType
ALU = mybir.AluOpType
AXIS = mybir.AxisListType

FLD = 40
NDELTA = 20
P = 780
HALF = 390

DVE_SLOTS = (4, 9, 14, 18, 22, 26, 30)   # rows (k mod 32) whose relu runs on DVE
BLOCKS = (4, 12, 24, 24, 24, 24, 16)


def build(nc, B_c=128, blocks=BLOCKS, dve_slots=DVE_SLOTS, n_warm=26):
    assert B_c == 128 and sum(blocks) == 128
    assert all(nb % 4 == 0 for nb in blocks)

    xTa_d = nc.dram_tensor("xTa", [128, B_c, 60], F16, kind="ExternalInput").ap()
    xTb_d = nc.dram_tensor("xTb", [128, B_c, 60], F16, kind="ExternalInput").ap()
    wT_d = nc.dram_tensor("wT", [128, 128], F16, kind="ExternalInput").ap()
    bias_d = nc.dram_tensor("bias", [128, 1], F32, kind="ExternalInput").ap()
    Zh_d = nc.dram_tensor("Zh", [128, 64], F16, kind="ExternalInput").ap()
    Zg_d = nc.dram_tensor("Zg", [128, 64], F16, kind="ExternalInput").ap()
    pb_d = nc.dram_tensor("pb", [128, 1], F32, kind="ExternalInput").ap()
    out_d = nc.dram_tensor("out", [4, 32], F32, kind="ExternalOutput").ap()
    scratch_d = nc.dram_tensor("scratch", [128, 8], F32, kind="Internal").ap()

    with tile.TileContext(nc) as tc:
        with (
            tc.tile_pool(name="const", bufs=1) as cpool,
            tc.tile_pool(name="hp", bufs=3) as hpool,
            tc.tile_pool(name="relu", bufs=20) as rpool,
            tc.tile_pool(name="awps", bufs=2, space="PSUM") as awpool,
            tc.tile_pool(name="accps", bufs=1, space="PSUM") as accpool,
        ):
            # ---- HAM pre-warm ----
            wsrc = cpool.tile([128, 256], F16, tag="wsrc")
            nc.vector.memset(wsrc[:], 0.0)
            outt = cpool.tile([128, 32], F32, tag="outt")
            nc.vector.memset(outt[:], 0.0)
            wps = accpool.tile([128, 512], F32, tag="sc_h0")  # reuse sc bank
            for i in range(n_warm):
                nc.tensor.matmul(wps[0:64, 0:256], wsrc[:, 0:64], wsrc[:],
                                 start=True, stop=True)

            xTa = cpool.tile([128, B_c, 60], F16, tag="xTa")
            xTb = cpool.tile([128, B_c, 60], F16, tag="xTb")
            wT_s = cpool.tile([128, 128], F16, tag="wT")
            bias_s = cpool.tile([128, 1], F32, tag="bias")
            Zh_s = cpool.tile([128, 64], F16, tag="Zh")
            Zg_s = cpool.tile([128, 64], F16, tag="Zg")
            pb_s = cpool.tile([128, 1], F32, tag="pb")

            # DMA issue order: first block + wT/bias first so compute can
            # start as early as possible (issues serialize on the sync queue).
            bounds = np.cumsum((0,) + blocks)
            def dma_block(t):
                b0, b1 = int(bounds[t]), int(bounds[t + 1])
                nc.sync.dma_start(xTa[:, b0:b1, :], xTa_d[:, b0:b1, :])
                nc.sync.dma_start(xTb[:, b0:b1, :], xTb_d[:, b0:b1, :])
            dma_block(0)
            nc.sync.dma_start(wT_s[:], wT_d[:])
            nc.sync.dma_start(bias_s[:], bias_d[:])
            dma_block(1)
            nc.sync.dma_start(Zh_s[:], Zh_d[:])
            nc.sync.dma_start(Zg_s[:], Zg_d[:])
            nc.sync.dma_start(pb_s[:], pb_d[:])

            sc_h1 = accpool.tile([128, 512], F32, tag="sc_h1")
            g_h0 = accpool.tile([128, 512], F32, tag="g_h0")
            g_h1 = accpool.tile([128, 512], F32, tag="g_h1")
            sc_h = [wps, sc_h1]
            g_h = [g_h0, g_h1]

            hp3_of = {}
            aw_of = {}
            relu_of = {}

            def hp_tt_ops(t):
                """Yield thunks, one per tensor_tensor of block t's hp build."""
                b0, b1 = int(bounds[t]), int(bounds[t + 1])
                NB = b1 - b0
                hp = hpool.tile([128, max(blocks) * P], F16, tag="hp",
                                name=f"hp{t}")
                hp3 = hp[:].rearrange("e (b q) -> e b q", q=P)
                for kk in range(NB):
                    hp3_of[b0 + kk] = (hp3, kk)
                def tt(d):
                    cnt = FLD if d < NDELTA else NDELTA
                    col0 = (d - 1) * FLD
                    if d % 2 == 0:
                        in1 = xTa[:, b0:b1, d:d + cnt]
                    else:
                        in1 = xTb[:, b0:b1, d - 1:d - 1 + cnt]
                    nc.vector.tensor_mul(
                        hp3[:, 0:NB, col0:col0 + cnt],
                        xTa[:, b0:b1, 0:cnt],
                        in1,
                    )
                return [(lambda d=d: tt(d)) for d in range(1, NDELTA + 1)]

            def emit_mm1(k):
                aw = awpool.tile([128, 1024], F32, tag="aw", name=f"aw{k}")
                aw_of[k] = aw
                hp3, kk = hp3_of[k]
                for h in (0, 1):
                    nc.tensor.matmul(
                        aw[:, 512 * h:512 * h + HALF],
                        wT_s[:],
                        hp3[:, kk, h * HALF:(h + 1) * HALF],
                        start=True, stop=True,
                    )

            def emit_relu(k):
                aw = aw_of.pop(k)
                relu = rpool.tile([128, P], F16, tag="relu", name=f"relu{k}")
                relu_of[k] = relu
                aw_v = aw[:].rearrange("a (u q) -> a u q", q=512)[:, :, 0:HALF]
                relu_v = relu[:].rearrange("a (u q) -> a u q", q=HALF)
                if (k % 32) in dve_slots:
                    # high priority: this relu gates the aw ping-pong; it must
                    # not queue behind bulk hp tensor_tensor work on the DVE
                    with tc.high_priority():
                        nc.vector.tensor_scalar(
                            out=relu_v, in0=aw_v,
                            scalar1=bias_s[:], scalar2=0.0,
                            op0=ALU.add, op1=ALU.max,
                        )
                else:
                    nc.scalar.activation(relu_v, aw_v, AF.Relu, bias=bias_s[:])

            def emit_scg(k0, nrows=8, sc_first=False):
                # rows k0..k0+nrows-1, emitted well after their relus so the
                # list scheduler keeps this batch contiguous (no mm1/scg
                # interleave transitions). Waves of 4 MMs hit 4 distinct
                # column groups and stream concurrently; g-waves first
                # (except the final batch, where sc finishing early lets the
                # softmax tail start under the last g-waves).
                def emit_g():
                    for q0 in range(k0, k0 + nrows, 4):
                        mps = [k // 4 for k in range(q0, q0 + 4)]
                        hps = [hp3_of.pop(k) for k in range(q0, q0 + 4)]
                        for h in (0, 1):
                            for j in range(4):
                                mp = mps[j]
                                hp3, kk = hps[j]
                                nc.tensor.matmul(
                                    g_h[h][32 * j:32 * j + 32, 0:HALF],
                                    Zg_s[:, 32 - mp:64 - mp],
                                    hp3[:, kk, h * HALF:(h + 1) * HALF],
                                    start=(mp == 0), stop=(mp == 31),
                                    tile_position=(0, 32 * j),
                                    skip_group_check=True,
                                )
                def emit_sc():
                    for q0 in range(k0, k0 + nrows, 4):
                        mps = [k // 4 for k in range(q0, q0 + 4)]
                        relus = [relu_of.pop(k) for k in range(q0, q0 + 4)]
                        for h in (0, 1):
                            for j in range(4):
                                mp = mps[j]
                                nc.tensor.matmul(
                                    sc_h[h][32 * j:32 * j + 32, 0:HALF],
                                    Zh_s[:, 32 - mp:64 - mp],
                                    relus[j][:, h * HALF:(h + 1) * HALF],
                                    start=(mp == 0), stop=(mp == 31),
                                    tile_position=(0, 32 * j),
                                    skip_group_check=True,
                                )
                if sc_first:
                    emit_sc(); emit_g()
                else:
                    emit_g(); emit_sc()

            # ---- main loop ----
            # Block t's DMA is issued during block t-2; block t's hp TTs are
            # interleaved into block t-1's row loop.
            nblocks = len(blocks)
            # fast start: rows 0-1's h0 matmuls depend only on pair-deltas
            # 1..10 (hp cols 0..400), so emit them before the remaining TTs
            tt0 = hp_tt_ops(0)
            for f in tt0[:10]:
                f()
            for k in (0, 1):
                aw = awpool.tile([128, 1024], F32, tag="aw", name=f"aw{k}")
                aw_of[k] = aw
                hp3, kk2 = hp3_of[k]
                nc.tensor.matmul(aw[:, 0:HALF], wT_s[:],
                                 hp3[:, kk2, 0:HALF], start=True, stop=True)
            for f in tt0[10:]:
                f()
            for k in (0, 1):
                hp3, kk2 = hp3_of[k]
                nc.tensor.matmul(aw_of[k][:, 512:512 + HALF], wT_s[:],
                                 hp3[:, kk2, HALF:2 * HALF],
                                 start=True, stop=True)
            emit_relu(0)
            emit_relu(1)
            pend_tt = hp_tt_ops(1) if nblocks > 1 else []

            for t in range(nblocks):
                b0, b1 = int(bounds[t]), int(bounds[t + 1])
                NB = b1 - b0
                if t + 2 < nblocks:
                    dma_block(t + 2)
                npairs = NB // 2
                tt_per_pair = (len(pend_tt) + npairs - 1) // max(npairs, 1)
                for kk in range(2 if t == 0 else 0, NB, 2):
                    k = b0 + kk
                    if k % 8 == 0 and k >= 16:
                        emit_scg(k - 16)
                    emit_mm1(k)
                    emit_mm1(k + 1)
                    emit_relu(k)
                    emit_relu(k + 1)
                    for _ in range(tt_per_pair):
                        if pend_tt:
                            pend_tt.pop(0)()
                while pend_tt:
                    pend_tt.pop(0)()
                pend_tt = hp_tt_ops(t + 2) if t + 2 < nblocks else []
            # DMA engine pre-warm: 8 tiny stores fired late (the memset lands
            # at the end of the DVE queue) so the HW DGE completion path is
            # awake when the output DMA issues.
            dmagate = cpool.tile([128, 1], F32, tag="dmagate")
            nc.vector.memset(dmagate[:], 0.0)
            for q in range(8):
                nc.sync.dma_start(scratch_d[:, q:q + 1], dmagate[:])

            emit_scg(112)
            emit_scg(120, sc_first=True)

            # ---- softmax tail ----
            exp_s = cpool.tile([128, P], F32, tag="exp_s")
            junk = cpool.tile([128, P], F32, tag="junk")
            negm = cpool.tile([128, 1], F32, tag="negm")
            denom = cpool.tile([128, 1], F32, tag="denom")
            rden = cpool.tile([128, 1], F32, tag="rden")
            numer = cpool.tile([128, 1], F32, tag="numer")
            outc = cpool.tile([128, 1], F32, tag="outc")

            negm2 = cpool.tile([128, 2], F32, tag="negm2")
            den2 = cpool.tile([128, 2], F32, tag="den2")
            for h in (0, 1):
                nc.vector.tensor_reduce(negm2[:, h:h + 1], sc_h[h][:, 0:HALF],
                                        axis=AXIS.X, op=ALU.max)
            nc.vector.tensor_reduce(negm[:], negm2[:], axis=AXIS.X,
                                    op=ALU.max, negate=True)
            for h in (0, 1):
                nc.scalar.activation(exp_s[:, h * HALF:(h + 1) * HALF],
                                     sc_h[h][:, 0:HALF], AF.Exp, bias=negm[:],
                                     accum_out=den2[:, h:h + 1])
                nc.vector.tensor_mul(junk[:, h * HALF:(h + 1) * HALF],
                                     exp_s[:, h * HALF:(h + 1) * HALF],
                                     g_h[h][:, 0:HALF])
            nc.vector.tensor_reduce(numer[:], junk[:], axis=AXIS.X, op=ALU.add)
            nc.vector.tensor_reduce(denom[:], den2[:], axis=AXIS.X, op=ALU.add)
            nc.vector.reciprocal(rden[:], denom[:])
            nc.vector.tensor_mul(outc[:], numer[:], rden[:])
            # gather the per-partition results into 4 partitions via a 32x32
            # block transpose so the output DMA is 4 descriptors, not 128
            nc.vector.tensor_scalar_add(outt[:, 0:1], outc[:], pb_s[:])
            outT = cpool.tile([128, 32], F32, tag="outT")
            nc.vector.transpose(outT[:], outt[:])
            for q in range(4):
                nc.sync.dma_start(out_d[q:q + 1, :],
                                  outT[32 * q:32 * q + 1, 0:32])

    nc.compile()
    return nc


def make_nc(B_c=128, blocks=BLOCKS):
    nc = bacc.Bacc("TRN2", target_bir_lowering=False, debug=False)
    build(nc, B_c=B_c, blocks=blocks)
    return nc


def perm_for(B_c=128, blocks=None):
    """perm[slot] = accumulator partition b for processing slot k."""
    return np.array([32 * (k % 4) + k // 4 for k in range(B_c)], np.int64)


def host_prep_consts(attn_w_w, attn_w_b, attn_h_w, attn_h_b, attn_p_w, attn_p_b):
    wT = np.ascontiguousarray(attn_w_w.T).astype(np.float16)
    bias = attn_w_b.reshape(128, 1).astype(np.float32)
    Zh = np.zeros((128, 64), np.float16)
    Zh[:, 32] = attn_h_w[0].astype(np.float16)
    Zg = np.zeros((128, 64), np.float16)
    Zg[:, 32] = attn_p_w[0].astype(np.float16)
    pb = np.full((128, 1), np.float32(attn_p_b[0]), np.float32)
    return {"wT": wT, "bias": bias, "Zh": Zh, "Zg": Zg, "pb": pb}


def host_prep_x(x_slice, blocks=None):
    # [B_c, F, E] -> two pre-shifted fp16 copies [E, B_c(perm), 60]
    xT = x_slice.transpose(2, 0, 1).astype(np.float16)
    xT = xT[:, perm_for(x_slice.shape[0]), :]
    B_c = x_slice.shape[0]
    xa = np.zeros((128, B_c, 60), np.float16)
    xa[:, :, 0:40] = xT
    xa[:, :, 40:60] = xT[:, :, 0:20]
    xb = np.zeros((128, B_c, 60), np.float16)
    xb[:, :, 0:59] = xa[:, :, 1:60]
    return np.ascontiguousarray(xa), np.ascontiguousarray(xb)


_NC_CACHE = {}


def _get_nc():
    if "nc" not in _NC_CACHE:
        _NC_CACHE["nc"] = make_nc()
    return _NC_CACHE["nc"]


def kernel(x, attn_w_w, attn_w_b, attn_h_w, attn_h_b, attn_p_w, attn_p_b,
           _trace=False):
    from concourse.bass_utils import run_bass_kernel_spmd
    x = np.asarray(x, np.float32)
    consts = host_prep_consts(np.asarray(attn_w_w), np.asarray(attn_w_b),
                              np.asarray(attn_h_w), np.asarray(attn_h_b),
                              np.asarray(attn_p_w), np.asarray(attn_p_b))
    in_maps = []
    for c in range(8):
        m = dict(consts)
        m["xTa"], m["xTb"] = host_prep_x(x[128 * c:128 * (c + 1)])
        in_maps.append(m)
    nc = _get_nc()
    res = run_bass_kernel_spmd(nc, in_maps, list(range(8)), trace=_trace)
    out = np.concatenate([res.results[c]["out"].reshape(128) for c in range(8)])
    if _trace:
        return out.astype(np.float32), res
    return out.astype(np.float32)


# revision 51
# speedup vs baseline: 1.2154x; 1.2154x over previous
"""Attentional Factorization Machine kernel for 8 Trainium2 NeuronCores.

Data-parallel over batch: 1024 rows -> 128 per core. Per core, per batch row:
  mm1: aw = W @ hp (hp = all 780 field-pair products, built on DVE)
  relu: ACT or DVE (split to balance engine load), bias fused
  scores + p_w-projection: one-hot stationary matmuls, issued as 4-wide waves
    across 4 distinct PE column groups so all four stream concurrently
  softmax over pairs + weighted combine on-chip in [128, 780] layout.

Structure notes:
- g-waves are emitted before sc-waves: g only needs hp (ready early), sc
  needs relu; this keeps the list scheduler from scrambling the group
  rotation that gives 4-way column concurrency.
- hp tensor_tensor builds for block t are interleaved into block t-1's row
  loop so the DVE queue never has a bulk burst blocking a due relu.
- HAM pre-warm: N=256 matmuls on a memset tile keep the PE busy from ~5us
  so the clock gate is open when real matmuls start.
"""
import sys
for _p in ("/opt/trn_rl_repo",):
    if _p not in sys.path:
        sys.path.insert(0, _p)

import numpy as np

import concourse.bass as bass
import concourse.bacc as bacc
import concourse.mybir as mybir
import concourse.tile as tile

F32 = mybir.dt.float32
F16 = mybir.dt.float16
AF = mybir.ActivationFunctionType
ALU = mybir.AluOpType
AXIS = mybir.AxisListType

FLD = 40
NDELTA = 20
P = 780
HALF = 390

DVE_SLOTS = (4, 9, 14, 18, 22, 26, 30)   # rows (k mod 32) whose relu runs on DVE
BLOCKS = (4, 12, 24, 24, 24, 24, 16)


def build(nc, B_c=128, blocks=BLOCKS, dve_slots=DVE_SLOTS, n_warm=22):
    assert B_c == 128 and sum(blocks) == 128
    assert all(nb % 4 == 0 for nb in blocks)

    xTa_d = nc.dram_tensor("xTa", [128, B_c, 60], F16, kind="ExternalInput").ap()
    xTb_d = nc.dram_tensor("xTb", [128, B_c, 60], F16, kind="ExternalInput").ap()
    wT_d = nc.dram_tensor("wT", [128, 128], F16, kind="ExternalInput").ap()
    bias_d = nc.dram_tensor("bias", [128, 1], F32, kind="ExternalInput").ap()
    Zh_d = nc.dram_tensor("Zh", [128, 64], F16, kind="ExternalInput").ap()
    Zg_d = nc.dram_tensor("Zg", [128, 64], F16, kind="ExternalInput").ap()
    pb_d = nc.dram_tensor("pb", [128, 1], F32, kind="ExternalInput").ap()
    out_d = nc.dram_tensor("out", [4, 32], F32, kind="ExternalOutput").ap()
    scratch_d = nc.dram_tensor("scratch", [128, 8], F32, kind="Internal").ap()

    with tile.TileContext(nc) as tc:
        with (
            tc.tile_pool(name="const", bufs=1) as cpool,
            tc.tile_pool(name="hp", bufs=3) as hpool,
            tc.tile_pool(name="relu", bufs=20) as rpool,
            tc.tile_pool(name="awps", bufs=2, space="PSUM") as awpool,
            tc.tile_pool(name="accps", bufs=1, space="PSUM") as accpool,
        ):
            # ---- HAM pre-warm ----
            wsrc = cpool.tile([128, 256], F16, tag="wsrc")
            nc.vector.memset(wsrc[:], 0.0)
            outt = cpool.tile([128, 32], F32, tag="outt")
            nc.vector.memset(outt[:], 0.0)
            wps = accpool.tile([128, 512], F32, tag="sc_h0")  # reuse sc bank
            for i in range(n_warm):
                nc.tensor.matmul(wps[0:64, 0:256], wsrc[:, 0:64], wsrc[:],
                                 start=True, stop=True)

            xTa = cpool.tile([128, B_c, 60], F16, tag="xTa")
            xTb = cpool.tile([128, B_c, 60], F16, tag="xTb")
            wT_s = cpool.tile([128, 128], F16, tag="wT")
            bias_s = cpool.tile([128, 1], F32, tag="bias")
            Zh_s = cpool.tile([128, 64], F16, tag="Zh")
            Zg_s = cpool.tile([128, 64], F16, tag="Zg")
            pb_s = cpool.tile([128, 1], F32, tag="pb")

            # DMA issue order: first block + wT/bias first so compute can
            # start as early as possible (issues serialize on the sync queue).
            bounds = np.cumsum((0,) + blocks)
            def dma_block(t):
                b0, b1 = int(bounds[t]), int(bounds[t + 1])
                nc.sync.dma_start(xTa[:, b0:b1, :], xTa_d[:, b0:b1, :])
                nc.sync.dma_start(xTb[:, b0:b1, :], xTb_d[:, b0:b1, :])
            dma_block(0)
            nc.sync.dma_start(wT_s[:], wT_d[:])
            nc.sync.dma_start(bias_s[:], bias_d[:])
            dma_block(1)
            nc.sync.dma_start(Zh_s[:], Zh_d[:])
            nc.sync.dma_start(Zg_s[:], Zg_d[:])
            nc.sync.dma_start(pb_s[:], pb_d[:])

            sc_h1 = accpool.tile([128, 512], F32, tag="sc_h1")
            g_h0 = accpool.tile([128, 512], F32, tag="g_h0")
            g_h1 = accpool.tile([128, 512], F32, tag="g_h1")
            sc_h = [wps, sc_h1]
            g_h = [g_h0, g_h1]

            hp3_of = {}
            aw_of = {}
            relu_of = {}

            def hp_tt_ops(t):
                """Yield thunks, one per tensor_tensor of block t's hp build."""
                b0, b1 = int(bounds[t]), int(bounds[t + 1])
                NB = b1 - b0
                hp = hpool.tile([128, max(blocks) * P], F16, tag="hp",
                                name=f"hp{t}")
                hp3 = hp[:].rearrange("e (b q) -> e b q", q=P)
                for kk in range(NB):
                    hp3_of[b0 + kk] = (hp3, kk)
                def tt(d):
                    cnt = FLD if d < NDELTA else NDELTA
                    col0 = (d - 1) * FLD
                    if d % 2 == 0:
                        in1 = xTa[:, b0:b1, d:d + cnt]
                    else:
                        in1 = xTb[:, b0:b1, d - 1:d - 1 + cnt]
                    nc.vector.tensor_mul(
                        hp3[:, 0:NB, col0:col0 + cnt],
                        xTa[:, b0:b1, 0:cnt],
                        in1,
                    )
                return [(lambda d=d: tt(d)) for d in range(1, NDELTA + 1)]

            def emit_mm1(k):
                aw = awpool.tile([128, 1024], F32, tag="aw", name=f"aw{k}")
                aw_of[k] = aw
                hp3, kk = hp3_of[k]
                for h in (0, 1):
                    nc.tensor.matmul(
                        aw[:, 512 * h:512 * h + HALF],
                        wT_s[:],
                        hp3[:, kk, h * HALF:(h + 1) * HALF],
                        start=True, stop=True,
                    )

            def emit_relu(k):
                aw = aw_of.pop(k)
                relu = rpool.tile([128, P], F16, tag="relu", name=f"relu{k}")
                relu_of[k] = relu
                aw_v = aw[:].rearrange("a (u q) -> a u q", q=512)[:, :, 0:HALF]
                relu_v = relu[:].rearrange("a (u q) -> a u q", q=HALF)
                if (k % 32) in dve_slots:
                    # high priority: this relu gates the aw ping-pong; it must
                    # not queue behind bulk hp tensor_tensor work on the DVE
                    with tc.high_priority():
                        nc.vector.tensor_scalar(
                            out=relu_v, in0=aw_v,
                            scalar1=bias_s[:], scalar2=0.0,
                            op0=ALU.add, op1=ALU.max,
                        )
                else:
                    nc.scalar.activation(relu_v, aw_v, AF.Relu, bias=bias_s[:])

            def emit_scg(k0, nrows=8, sc_first=False):
                # rows k0..k0+nrows-1, emitted well after their relus so the
                # list scheduler keeps this batch contiguous (no mm1/scg
                # interleave transitions). Waves of 4 MMs hit 4 distinct
                # column groups and stream concurrently; g-waves first
                # (except the final batch, where sc finishing early lets the
                # softmax tail start under the last g-waves).
                def emit_g():
                    for q0 in range(k0, k0 + nrows, 4):
                        mps = [k // 4 for k in range(q0, q0 + 4)]
                        hps = [hp3_of.pop(k) for k in range(q0, q0 + 4)]
                        for h in (0, 1):
                            for j in range(4):
                                mp = mps[j]
                                hp3, kk = hps[j]
                                nc.tensor.matmul(
                                    g_h[h][32 * j:32 * j + 32, 0:HALF],
                                    Zg_s[:, 32 - mp:64 - mp],
                                    hp3[:, kk, h * HALF:(h + 1) * HALF],
                                    start=(mp == 0), stop=(mp == 31),
                                    tile_position=(0, 32 * j),
                                    skip_group_check=True,
                                )
                def emit_sc():
                    for q0 in range(k0, k0 + nrows, 4):
                        mps = [k // 4 for k in range(q0, q0 + 4)]
                        relus = [relu_of.pop(k) for k in range(q0, q0 + 4)]
                        for h in (0, 1):
                            for j in range(4):
                                mp = mps[j]
                                nc.tensor.matmul(
                                    sc_h[h][32 * j:32 * j + 32, 0:HALF],
                                    Zh_s[:, 32 - mp:64 - mp],
                                    relus[j][:, h * HALF:(h + 1) * HALF],
                                    start=(mp == 0), stop=(mp == 31),
                                    tile_position=(0, 32 * j),
                                    skip_group_check=True,
                                )
                if sc_first:
                    emit_sc(); emit_g()
                else:
                    emit_g(); emit_sc()

            # ---- main loop ----
            # Block t's DMA is issued during block t-2; block t's hp TTs are
            # interleaved into block t-1's row loop.
            nblocks = len(blocks)
            # fast start: rows 0-1's h0 matmuls depend only on pair-deltas
            # 1..10 (hp cols 0..400), so emit them before the remaining TTs
            tt0 = hp_tt_ops(0)
            for f in tt0[:10]:
                f()
            for k in (0, 1):
                aw = awpool.tile([128, 1024], F32, tag="aw", name=f"aw{k}")
                aw_of[k] = aw
                hp3, kk2 = hp3_of[k]
                nc.tensor.matmul(aw[:, 0:HALF], wT_s[:],
                                 hp3[:, kk2, 0:HALF], start=True, stop=True)
            for f in tt0[10:]:
                f()
            for k in (0, 1):
                hp3, kk2 = hp3_of[k]
                nc.tensor.matmul(aw_of[k][:, 512:512 + HALF], wT_s[:],
                                 hp3[:, kk2, HALF:2 * HALF],
                                 start=True, stop=True)
            emit_relu(0)
            emit_relu(1)
            pend_tt = hp_tt_ops(1) if nblocks > 1 else []

            for t in range(nblocks):
                b0, b1 = int(bounds[t]), int(bounds[t + 1])
                NB = b1 - b0
                if t + 2 < nblocks:
                    dma_block(t + 2)
                npairs = NB // 2
                tt_per_pair = (len(pend_tt) + npairs - 1) // max(npairs, 1)
                for kk in range(2 if t == 0 else 0, NB, 2):
                    k = b0 + kk
                    if k % 8 == 0 and k >= 16:
                        emit_scg(k - 16)
                    emit_mm1(k)
                    emit_mm1(k + 1)
                    emit_relu(k)
                    emit_relu(k + 1)
                    for _ in range(tt_per_pair):
                        if pend_tt:
                            pend_tt.pop(0)()
                while pend_tt:
                    pend_tt.pop(0)()
                pend_tt = hp_tt_ops(t + 2) if t + 2 < nblocks else []
            # DMA engine pre-warm: 8 tiny stores fired late (the memset lands
            # at the end of the DVE queue) so the HW DGE completion path is
            # awake when the output DMA issues.
            dmagate = cpool.tile([128, 1], F32, tag="dmagate")
            nc.vector.memset(dmagate[:], 0.0)
            for q in range(8):
                nc.sync.dma_start(scratch_d[:, q:q + 1], dmagate[:])

            emit_scg(112)
            emit_scg(120, sc_first=True)

            # ---- softmax tail ----
            exp_s = cpool.tile([128, P], F32, tag="exp_s")
            junk = cpool.tile([128, P], F32, tag="junk")
            negm = cpool.tile([128, 1], F32, tag="negm")
            denom = cpool.tile([128, 1], F32, tag="denom")
            rden = cpool.tile([128, 1], F32, tag="rden")
            numer = cpool.tile([128, 1], F32, tag="numer")
            outc = cpool.tile([128, 1], F32, tag="outc")

            negm2 = cpool.tile([128, 2], F32, tag="negm2")
            den2 = cpool.tile([128, 2], F32, tag="den2")
            for h in (0, 1):
                nc.vector.tensor_reduce(negm2[:, h:h + 1], sc_h[h][:, 0:HALF],
                                        axis=AXIS.X, op=ALU.max)
            nc.vector.tensor_reduce(negm[:], negm2[:], axis=AXIS.X,
                                    op=ALU.max, negate=True)
            for h in (0, 1):
                nc.scalar.activation(exp_s[:, h * HALF:(h + 1) * HALF],
                                     sc_h[h][:, 0:HALF], AF.Exp, bias=negm[:],
                                     accum_out=den2[:, h:h + 1])
                nc.vector.tensor_mul(junk[:, h * HALF:(h + 1) * HALF],
                                     exp_s[:, h * HALF:(h + 1) * HALF],
                                     g_h[h][:, 0:HALF])
            nc.vector.tensor_reduce(numer[:], junk[:], axis=AXIS.X, op=ALU.add)
            nc.vector.tensor_reduce(denom[:], den2[:], axis=AXIS.X, op=ALU.add)
            nc.vector.reciprocal(rden[:], denom[:])
            nc.vector.tensor_mul(outc[:], numer[:], rden[:])
            # gather the per-partition results into 4 partitions via a 32x32
            # block transpose so the output DMA is 4 descriptors, not 128
            nc.vector.tensor_scalar_add(outt[:, 0:1], outc[:], pb_s[:])
            outT = cpool.tile([128, 32], F32, tag="outT")
            nc.vector.transpose(outT[:], outt[:])
            nc.sync.dma_start(out_d[:], outT[0:128:32, 0:32])

    nc.compile()
    return nc


def make_nc(B_c=128, blocks=BLOCKS):
    nc = bacc.Bacc("TRN2", target_bir_lowering=False, debug=False)
    build(nc, B_c=B_c, blocks=blocks)
    return nc


def perm_for(B_c=128, blocks=None):
    """perm[slot] = accumulator partition b for processing slot k."""
    return np.array([32 * (k % 4) + k // 4 for k in range(B_c)], np.int64)


def host_prep_consts(attn_w_w, attn_w_b, attn_h_w, attn_h_b, attn_p_w, attn_p_b):
    wT = np.ascontiguousarray(attn_w_w.T).astype(np.float16)
    bias = attn_w_b.reshape(128, 1).astype(np.float32)
    Zh = np.zeros((128, 64), np.float16)
    Zh[:, 32] = attn_h_w[0].astype(np.float16)
    Zg = np.zeros((128, 64), np.float16)
    Zg[:, 32] = attn_p_w[0].astype(np.float16)
    pb = np.full((128, 1), np.float32(attn_p_b[0]), np.float32)
    return {"wT": wT, "bias": bias, "Zh": Zh, "Zg": Zg, "pb": pb}


def host_prep_x(x_slice, blocks=None):
    # [B_c, F, E] -> two pre-shifted fp16 copies [E, B_c(perm), 60]
    xT = x_slice.transpose(2, 0, 1).astype(np.float16)
    xT = xT[:, perm_for(x_slice.shape[0]), :]
    B_c = x_slice.shape[0]
    xa = np.zeros((128, B_c, 60), np.float16)
    xa[:, :, 0:40] = xT
    xa[:, :, 40:60] = xT[:, :, 0:20]
    xb = np.zeros((128, B_c, 60), np.float16)
    xb[:, :, 0:59] = xa[:, :, 1:60]
    return np.ascontiguousarray(xa), np.ascontiguousarray(xb)


_NC_CACHE = {}


def _get_nc():
    if "nc" not in _NC_CACHE:
        _NC_CACHE["nc"] = make_nc()
    return _NC_CACHE["nc"]


def kernel(x, attn_w_w, attn_w_b, attn_h_w, attn_h_b, attn_p_w, attn_p_b,
           _trace=False):
    from concourse.bass_utils import run_bass_kernel_spmd
    x = np.asarray(x, np.float32)
    consts = host_prep_consts(np.asarray(attn_w_w), np.asarray(attn_w_b),
                              np.asarray(attn_h_w), np.asarray(attn_h_b),
                              np.asarray(attn_p_w), np.asarray(attn_p_b))
    in_maps = []
    for c in range(8):
        m = dict(consts)
        m["xTa"], m["xTb"] = host_prep_x(x[128 * c:128 * (c + 1)])
        in_maps.append(m)
    nc = _get_nc()
    res = run_bass_kernel_spmd(nc, in_maps, list(range(8)), trace=_trace)
    out = np.concatenate([res.results[c]["out"].reshape(128) for c in range(8)])
    if _trace:
        return out.astype(np.float32), res
    return out.astype(np.float32)
